# revision 58
# baseline (speedup 1.0000x reference)
"""Self-contained Trainium2 Bass kernel for nn_AttLayer_model_5.

kernel(**inputs) takes the FULL unsharded inputs (B=64, T=2048, D=256, H=5),
shards the batch across 8 NeuronCores (data-parallel, 8 samples/core),
runs a Bass/Tile kernel via concourse.bass_utils.run_bass_kernel_spmd,
and gathers the full (64, 256) float32 output.

Math (per sample):
  temp  = x @ W_temp + b_temp          # (T,H), contraction over D
  fea   = xfea[:,None]*W_fea[0] + b_fea
  had   = tanh(temp) * tanh(fea)
  inter = had @ v, v = uw.sum(1)       # sum(b) shift dropped: softmax-invariant
  e     = exp(inter)                   # no max-subtraction: |inter| is bounded
                                       # by sum_h |v_h| ~ 0.5, fp32-safe
  wnum  = e * mask
  y     = (wnum @ x) / sum(wnum)       # (D,)

Device strategy (per core, 8 samples, x shard shipped bf16 = 8 MiB):
- x is cast to bf16 on host: every on-device x consumer (PE transposes at
  1 cyc/col vs 2 for fp32, projection, pooling matmuls at 1 cyc/col vs 4)
  runs at 16-bit PE rates and HBM traffic halves. End-to-end rel err vs
  the fp32 reference is ~2.4e-3 (gate is 2e-2): pooling error ~bf16 eps
  dampened by averaging; projection error further damped by
  d(inter)/d(temp) ~ 5e-3.
- x resident in SBUF, token-partition layout t = 16*p + c (contiguous DMA
  bursts), 4 quarter-DMAs per sample on the SP HWDGE queue in consumption
  order. Constants ride one packed [128,594] f32 tensor + one packed
  [12,4096] bf16 xfea/mask tensor on the ACT HWDGE queue -- any SWDGE
  (gpsimd) use costs ~0.4 ms/execute of setup on real HW.
- D-contraction transposes: both 128-row halves of a sample-stripe share
  one full-bank [128,1024] bf16 PSUM tile (transpose-mode matmuls), one
  psum->sbuf copy per sample-stripe, rotated DVE/DVE/ACT.
- Projection packs 4 samples per PSUM tile at partition offsets 32*j via
  matmul column tiling; biases ride ACT bias patterns; inter comes from a
  v-folding pattern matmul; softmax runs in an (8, T) layout; phases are
  group-major (samples 0-3 across all stripes, then 4-7), and the PE
  stream is software-pipelined one group deep: each group's projection/
  tanh chain is emitted behind the next group's independent transposes,
  hiding the ~us-scale real cross-engine semaphore latency that would
  otherwise head-of-line-block the in-order PE queue.
- Pooling: per 128-token chunk, wnum columns are PE-transposed and fed to
  M=1 bf16 matmuls (4 samples column-packed) accumulating fp32 in PSUM
  over 16 chunks, deferred one stripe to fill phase-B stalls; 1/sum(wnum)
  lands in a bulk per-partition-scalar scaled gather of the (pre-zeroed)
  accumulator banks -> 2 partition-strided y DMAs.
- The whole body is replicated K_IN times inside one program (constants
  loaded once, x re-read from HBM each iteration, y rewritten): one
  device execute performs K_IN genuine kernel executions, amortizing the
  per-execute axon dispatch cost so steady-state per-iteration hardware
  time is measurable from wall-clock slopes. x tiles, e/softmax tiles and
  output staging are double-buffered so iteration i+1's HBM reads and
  phase A overlap iteration i's phase B/pooling.

Cost-model per-core time: 119.5 us (fp32 baseline) -> 44.8 us/iteration.
Measured per-iteration HW time (8 cores concurrent, slope method):
~230-310 us depending on shared-terminal load; the x-read DMA floor
alone measures ~95-130 us/iteration in the same environment.
"""

import os
import sys
from contextlib import ExitStack

import numpy as np

for _p in ("/opt/trn_rl_repo", "/root/.axon_site/_ro/trn_rl_repo"):
    if os.path.isdir(_p) and _p not in sys.path:
        sys.path.insert(0, _p)
        break

import concourse.bass as bass
import concourse.mybir as mybir
import concourse.tile as tile
from concourse import bacc
from concourse.bass_utils import run_bass_kernel_spmd

F32 = mybir.dt.float32
BF16 = mybir.dt.bfloat16

N_CORES = 8
B = 64
B_LOC = B // N_CORES  # 8 samples per core
T = 2048
D = 256
H = 5
NC16 = T // 128
NQ = T // 512
K_IN = int(os.environ.get("BASS_K_IN", "64"))  # on-device kernel iterations per execute
# phase-bisection variants for HW attribution: full | dma | transp | proj
_VARIANT = os.environ.get("BASS_VARIANT", "full")
AF = mybir.ActivationFunctionType
ALU = mybir.AluOpType

# packed-constant column offsets in cpak [128, CPAK_COLS] f32
_IDENT0, _WT0, _VPAT0, _BT0, _BF0, _FPAT0, _PATG0 = 0, 128, 192, 208, 209, 210, 338
CPAK_COLS = 338 + 256


def _host_constants(W_temp, b_temp, W_fea, b_fea, uw):
    """Pure O(D*H + H^2) weight repacking on host into one tensor."""
    W_temp = np.asarray(W_temp, np.float32)
    b_temp = np.asarray(b_temp, np.float32)
    W_fea = np.asarray(W_fea, np.float32)
    b_fea = np.asarray(b_fea, np.float32)
    uw = np.asarray(uw, np.float32)

    v = uw.sum(axis=1)

    cpak = np.zeros((128, CPAK_COLS), np.float32)
    cpak[:, _IDENT0 : _IDENT0 + 128] = np.eye(128, dtype=np.float32)
    # wt: [128, 64], D halves at col offsets 0/32
    cpak[:, _WT0 : _WT0 + H] = W_temp[:128]
    cpak[:, _WT0 + 32 : _WT0 + 32 + H] = W_temp[128:]
    for s in range(B_LOC):
        g, j = divmod(s, 4)
        cpak[32 * j : 32 * j + H, _VPAT0 + 8 * g + s] = v
    for j in range(4):
        cpak[32 * j : 32 * j + H, _BT0] = b_temp
        cpak[32 * j : 32 * j + H, _BF0] = b_fea
        cpak[j, _FPAT0 + 32 * j : _FPAT0 + 32 * j + H] = W_fea[0]
    for g in range(2):
        for j in range(4):
            cpak[4 * g + j, _PATG0 + 128 * g + 32 * j] = 1.0
    return cpak


def _declare_io(nc):
    io = {}
    io["x"] = nc.dram_tensor("x", [B_LOC, T, D], BF16, kind="ExternalInput")
    # xm rows 0-3: xfea [4, 2T]; rows 4-11: mask-units [8, T] (cols T: pad)
    io["xm"] = nc.dram_tensor("xm", [12, 2 * T], BF16, kind="ExternalInput")
    io["cpak"] = nc.dram_tensor("cpak", [128, CPAK_COLS], F32, kind="ExternalInput")
    # unused pad input: forces HLO-hash/compile-cache misses so every
    # build of this program is compiled fresh (cache-buster, never read).
    # Keyed by K_IN and variant: program variants share I/O shapes, and a
    # stale HLO-hash hit would silently load the wrong NEFF.
    voff = {"full": 0, "dma": 1, "transp": 2, "proj": 3, "poolnf": 4, "poolend": 5}[
        _VARIANT
    ]
    io["pad"] = nc.dram_tensor(
        "pad", [1, 16 + K_IN + 512 * voff], F32, kind="ExternalInput"
    )
    io["y"] = nc.dram_tensor("y", [B_LOC, D], F32, kind="ExternalOutput")
    return io


class _Consts:
    pass


def _build_consts(nc, tc, io, ctx):
    """One-time loads/casts + pool creation shared by all body iterations."""
    C = _Consts()
    # All const loads ride the ACT HWDGE queue: any SWDGE (gpsimd) use
    # costs ~ms of per-execute setup on real HW (ucode library load +
    # software descriptor generation)
    cpool = ctx.enter_context(tc.tile_pool(name="consts", bufs=1))
    cpak = cpool.tile([128, CPAK_COLS], F32, name="cpak_sb")
    nc.scalar.dma_start(cpak[:], io["cpak"].ap()[:])
    xfea_h = cpool.tile([4, 2 * T], BF16, name="xfea_sb")
    nc.scalar.dma_start(xfea_h[:], io["xm"].ap()[0:4, :])
    masku_b = cpool.tile([B_LOC, T], BF16, name="masku_b")
    nc.scalar.dma_start(masku_b[:], io["xm"].ap()[4:12, 0:T])
    C.masku = cpool.tile([B_LOC, T], F32, name="masku_f")
    nc.vector.tensor_copy(C.masku[:], masku_b[:])

    C.ident_sb = cpak[:, _IDENT0 : _IDENT0 + 128]
    C.btpat = cpak[:, _BT0 : _BT0 + 1]
    C.bfpat = cpak[:, _BF0 : _BF0 + 1]
    C.patg = cpak[0:8, _PATG0 : _PATG0 + 256]
    C.xfea_h = xfea_h[:]

    # bf16 casts of the stationary matmul operands (fp32 can't pair with
    # bf16 on the PE)
    C.ident_h = cpool.tile([128, 128], BF16, name="ident_h")
    nc.vector.tensor_copy(C.ident_h[:], C.ident_sb)
    C.wt_h = cpool.tile([128, 64], BF16, name="wt_h")
    nc.vector.tensor_copy(C.wt_h[:], cpak[:, _WT0 : _WT0 + 64])
    C.vpat_h = cpool.tile([128, 16], BF16, name="vpat_h")
    nc.vector.tensor_copy(C.vpat_h[:], cpak[:, _VPAT0 : _VPAT0 + 16])
    C.fpat_h = cpool.tile([4, 128], BF16, name="fpat_h")
    nc.vector.tensor_copy(C.fpat_h[:], cpak[0:4, _FPAT0 : _FPAT0 + 128])

    # double-buffered across body iterations: x DMA and phase A of
    # iteration i+1 overlap phase B / pooling of iteration i
    C.xpool = ctx.enter_context(tc.tile_pool(name="xres", bufs=2))
    C.e_pool = ctx.enter_context(tc.tile_pool(name="epool", bufs=2))
    C.xtp_pool = ctx.enter_context(tc.tile_pool(name="xtp", bufs=3, space="PSUM"))
    C.xts_pool = ctx.enter_context(tc.tile_pool(name="xts", bufs=9))
    C.ttp_pool = ctx.enter_context(tc.tile_pool(name="ttp", bufs=1, space="PSUM"))
    C.fi_pool = ctx.enter_context(tc.tile_pool(name="fi", bufs=2, space="PSUM"))
    C.act_pool = ctx.enter_context(tc.tile_pool(name="acts", bufs=2))
    C.p3_pool = ctx.enter_context(tc.tile_pool(name="p3", bufs=1, space="PSUM"))
    C.out_pool = ctx.enter_context(tc.tile_pool(name="outp", bufs=2))
    return C


def _build_body(nc, tc, io, C, it):
    """One full kernel iteration: x HBM read -> compute -> y write."""
    mm = nc.tensor.matmul

    # x tiles: 4 quarter-DMAs per sample on the SP HWDGE queue, emitted in
    # the order the group-major phases consume them
    x_sb = [
        C.xpool.tile([128, NC16 * D], BF16, name=f"x{it}_{s}", tag=f"x{s}")
        for s in range(B_LOC)
    ]
    # one full-sample DMA each: the (c d) free dim is one contiguous 8 KiB
    # DRAM run per partition, so this emits 128 descriptors per sample
    # (4x fewer than quarter-DMAs; the real DMA path is descriptor-rate
    # bound). Arrival granularity is hidden by the cross-iteration double
    # buffering of x.
    for s in range(B_LOC):
        src = io["x"].ap()[s].rearrange("(p c) d -> p (c d)", c=NC16)
        nc.sync.dma_start(x_sb[s][:], src)

    do_proj = _VARIANT in ("proj", "full", "poolnf", "poolend")
    do_pool = _VARIANT in ("full", "poolnf", "poolend")
    defer_pool = _VARIANT != "poolend"  # poolend: all pooling after phase B
    do_finale = _VARIANT in ("full", "poolend")

    def y_stub():
        ystage = C.out_pool.tile([1, B_LOC * D], F32, name=f"ys{it}", tag="ystage")
        nc.vector.memset(ystage[:], 0.0)
        nc.sync.dma_start(io["y"].ap().rearrange("s d -> () (s d)"), ystage[:])

    if _VARIANT == "dma":
        y_stub()
        return

    # phase-3 accumulators: wtp and ypp0 share one bank-tile, ypp1 its own.
    # The banks are zeroed up front: the M=1 pooling matmuls only write the
    # 32*j sample bands, but the final scaled gather bulk-reads all rows.
    combo = C.p3_pool.tile([128, 512], F32, name=f"combo{it}", tag="combo")
    wtp = combo[:, 0:128]
    ypps = [
        combo[:, 128:384],
        C.p3_pool.tile([128, D], F32, name=f"ypp1_{it}", tag="ypp1"),
    ]
    recp = combo[:, 384:386]
    if do_pool:
        nc.vector.memset(combo[:, 128:386], 0.0)
        nc.vector.memset(ypps[1][:], 0.0)
    wts = C.out_pool.tile([128, 128], BF16, name=f"wts{it}", tag="wts")

    e_sb = C.e_pool.tile([B_LOC, T], F32, name=f"e{it}", tag="e_sb")
    den4_sb = C.e_pool.tile([B_LOC, NQ], F32, name=f"d4{it}", tag="den4")
    den_sb = C.e_pool.tile([B_LOC, 1], F32, name=f"d{it}", tag="den")
    rec_sb = C.e_pool.tile([B_LOC, 1], F32, name=f"r{it}", tag="rec")

    copy_flip = [0]

    def psum_to_sbuf(dst, src):
        # DVE copies bf16 ~1.6x faster than ACT; weight the rotation 2:1
        if copy_flip[0] % 3 != 2:
            nc.vector.tensor_copy(dst, src)
        else:
            nc.scalar.copy(dst, src)
        copy_flip[0] += 1

    # tanh(fea) stripes precomputed as PE filler during the x DMA wait
    tfs_all = {}

    def emit_tfs(q, g):
        fep = C.fi_pool.tile([128, 512], F32, name=f"fp{it}_{q}{g}", tag="fi")
        mm(fep[:], C.fpat_h[:], C.xfea_h[:, bass.ds(g * T + 512 * q, 512)])
        tfs = C.act_pool.tile(
            [128, 512], BF16, name=f"tf{it}_{q}{g}", tag="tfs", bufs=8
        )
        nc.scalar.activation(tfs[:], fep[:], AF.Tanh, bias=C.bfpat)
        tfs_all[(q, g)] = tfs

    tfs_todo = [(q, g) for g in range(2) for q in range(NQ)]

    def transp_group(q, g):
        """transposes + psum->sbuf(bf16) copies for one sample group.

        Both 128-row halves of the D contraction share one full-bank
        [128,1024] bf16 PSUM tile per sample, so 3 PSUM bufs hold 3
        samples in flight and each sample needs a single copy."""
        xts_h = {}
        for j in range(4):
            s = 4 * g + j
            xtp = C.xtp_pool.tile(
                [128, 1024], BF16, name=f"xp{it}_{q}{s}", tag="xtp"
            )
            for dh in range(2):
                for i in range(4):
                    c = 4 * q + i
                    mm(
                        xtp[:, 512 * dh + 128 * i : 512 * dh + 128 * (i + 1)],
                        x_sb[s][:, bass.ds(c * D + dh * 128, 128)],
                        C.ident_h[:],
                        is_transpose=True,
                        start=(dh == 0 and i == 0),
                        stop=(dh == 1 and i == 3),
                    )
            xts = C.xts_pool.tile(
                [128, 1024], BF16, name=f"xs{it}_{q}{s}", tag="xts"
            )
            psum_to_sbuf(xts[:], xtp[:])
            xts_h[j] = xts
            if do_proj and j == 1 and tfs_todo:
                emit_tfs(*tfs_todo.pop(0))
        if do_proj and tfs_todo:
            emit_tfs(*tfs_todo.pop(0))
        return xts_h

    def proj_mms(q, g, xts_h):
        ttp = C.ttp_pool.tile([128, 512], F32, name=f"tt{it}_{q}{g}", tag="ttp")
        for dh in range(2):
            for j in range(4):
                mm(
                    ttp[32 * j : 32 * j + 32, :],
                    C.wt_h[:, 32 * dh : 32 * dh + 32],
                    xts_h[j][:, 512 * dh : 512 * dh + 512],
                    start=(dh == 0),
                    stop=(dh == 1),
                    tile_position=(0, 32 * j),
                    skip_group_check=True,
                )
        return ttp

    def tanh_had_v(q, g, ttp):
        """tanh(temp), hadamard with precomputed tanh(fea), V-matmul."""
        tts = C.act_pool.tile([128, 512], F32, name=f"ts{it}_{q}{g}", tag="tts")
        nc.scalar.activation(tts[:], ttp[:], AF.Tanh, bias=C.btpat)
        had = C.act_pool.tile([128, 512], BF16, name=f"hd{it}_{q}{g}", tag="had")
        nc.vector.tensor_mul(had[:], tts[:], tfs_all[(q, g)][:])
        itp = C.fi_pool.tile([128, 512], F32, name=f"it{it}_{q}{g}", tag="fi")
        mm(itp[:8, :], C.vpat_h[:, 8 * g : 8 * g + 8], had[:])
        return itp

    def pool_wts(q):
        """w-transposes + psum->sbuf copy for stripe q, emitted right after
        its exp so the copy lands before the pooling MMs need it."""
        for i in range(4):
            c = 4 * q + i
            mm(
                wtp[:, 8 * c : 8 * c + 8],
                e_sb[:, 128 * c : 128 * (c + 1)],
                C.ident_sb[:8, :8],
                is_transpose=True,
                start=(c == 0),
                stop=(c == NC16 - 1),
                skip_group_check=True,
            )
        psum_to_sbuf(wts[:, 32 * q : 32 * (q + 1)], wtp[:, 32 * q : 32 * (q + 1)])

    def pool_mms(q):
        """packed bf16 pooling MMs for stripe q (one group behind pool_wts)."""
        for i in range(4):
            c = 4 * q + i
            for g in range(2):
                for j in range(4):
                    s = 4 * g + j
                    mm(
                        ypps[g][32 * j : 32 * j + 1, :],
                        wts[:, 8 * c + s : 8 * c + s + 1],
                        x_sb[s][:, bass.ds(c * D, D)],
                        start=(c == 0),
                        stop=(c == NC16 - 1),
                        tile_position=(0, 32 * j),
                        skip_group_check=True,
                    )

    def pool_stripe(q):
        pool_wts(q)
        pool_mms(q)

    def finish_group(q, g, xts_h):
        """projection + tanh/hadamard/V chain for a group whose transposes
        were emitted one group earlier (keeps independent transposes ahead
        of dependency-waiting matmuls in the in-order PE queue)."""
        ttp = proj_mms(q, g, xts_h)
        if g == 1 and do_pool and defer_pool and q >= 1:
            pool_mms(q - 1)
        itp = tanh_had_v(q, g, ttp)
        if g == 0:
            nc.vector.tensor_add(
                e_sb[:, bass.ds(512 * q, 512)],
                itp[:8, :],
                C.masku[:, bass.ds(512 * q, 512)],
            )
        else:
            inter = C.act_pool.tile([8, 512], F32, name=f"in{it}_{q}", tag="inter")
            nc.vector.tensor_add(
                inter[:], itp[:8, :], e_sb[:, bass.ds(512 * q, 512)]
            )
            nc.scalar.activation(
                e_sb[:, bass.ds(512 * q, 512)],
                inter[:],
                AF.Exp,
                accum_out=den4_sb[:, q : q + 1],
            )
            if do_pool and defer_pool:
                pool_wts(q)

    # ---- phases, PE stream software-pipelined one group deep:
    # group 0 (samples 0-3) across all stripes, then group 1 (4-7) ----
    pend = None
    for g in range(2):
        for q in range(NQ):
            xts_h = transp_group(q, g)
            if do_proj and pend is not None:
                finish_group(*pend)
            pend = (q, g, xts_h)
    if do_proj:
        finish_group(*pend)
    if not do_pool:
        y_stub()
        return
    if defer_pool:
        pool_mms(NQ - 1)
    else:
        for q in range(NQ):
            pool_stripe(q)
    if not do_finale:
        y_stub()
        return

    # ---- finale: denominators -> reciprocal patterns -> scaled gather ----
    nc.vector.tensor_reduce(
        den_sb[:], den4_sb[:], axis=mybir.AxisListType.X, op=ALU.add
    )
    nc.vector.reciprocal(rec_sb[:], den_sb[:])
    for g in range(2):
        mm(recp[:, g : g + 1], C.patg[:, 128 * g : 128 * (g + 1)], rec_sb[:])
    recs = C.out_pool.tile([128, 2], F32, name=f"rc{it}", tag="recs")
    nc.vector.tensor_copy(recs[:], recp[:])

    # scaled gather: one per-partition-scalar multiply per group (rows off
    # the 32*j sample bands scale by zero), then partition-strided y DMAs
    ystage = C.out_pool.tile([128, 2 * D], F32, name=f"ys{it}", tag="ystage")
    nc.vector.tensor_scalar_mul(ystage[:, 0:D], ypps[0][:, :], recs[:, 0:1])
    nc.scalar.mul(ystage[:, D : 2 * D], ypps[1][:, :], recs[:, 1:2])
    for g in range(2):
        src = ystage[:, g * D : (g + 1) * D].rearrange(
            "(j p) d -> j p d", p=32
        )[:, 0, :]
        nc.sync.dma_start(io["y"].ap()[4 * g : 4 * g + 4, :], src)


def _build(nc, tc, io, ctx):
    C = _build_consts(nc, tc, io, ctx)
    for it in range(K_IN):
        _build_body(nc, tc, io, C, it)


_MODULE_CACHE = {}


def _get_module():
    if "nc" not in _MODULE_CACHE:
        nc = bacc.Bacc("TRN2", target_bir_lowering=False, debug=False)
        io = _declare_io(nc)
        with tile.TileContext(nc) as tc:
            with ExitStack() as ctx:
                _build(nc, tc, io, ctx)
        nc.compile()
        _MODULE_CACHE["nc"] = nc
    return _MODULE_CACHE["nc"]


def make_in_maps(x_temp, x_fea, mask, W_temp, b_temp, W_fea, b_fea, b, uw):
    """Shard full inputs into per-core input maps (host-side, O(bytes))."""
    import ml_dtypes

    bf = ml_dtypes.bfloat16
    x_temp = np.ascontiguousarray(np.asarray(x_temp, np.float32).astype(bf))
    x_fea = np.asarray(x_fea, np.float32)
    masku = np.asarray(mask).astype(np.uint8)
    cpak = _host_constants(W_temp, b_temp, W_fea, b_fea, uw)

    in_maps = []
    for k in range(N_CORES):
        sl = slice(k * B_LOC, (k + 1) * B_LOC)
        # on-chip token order: free position 128*c + p <-> token 16*p + c
        xfea_p = (
            x_fea[sl].reshape(B_LOC, 128, NC16).swapaxes(1, 2).reshape(B_LOC, T)
        )
        xm = np.zeros((12, 2 * T), np.float32)
        xm[0:4] = xfea_p.reshape(2, 4, T).swapaxes(0, 1).reshape(4, 2 * T)
        xm[4:12, 0:T] = np.where(
            masku[sl].reshape(B_LOC, 128, NC16).swapaxes(1, 2).reshape(B_LOC, T)
            != 0,
            np.float32(0.0),
            np.float32(-1e30),
        )
        in_maps.append(
            {
                "pad": np.zeros(
                    (
                        1,
                        16
                        + K_IN
                        + 512
                        * {
                            "full": 0,
                            "dma": 1,
                            "transp": 2,
                            "proj": 3,
                            "poolnf": 4,
                            "poolend": 5,
                        }[_VARIANT],
                    ),
                    np.float32,
                ),
                "x": x_temp[sl],
                "xm": xm.astype(bf),
                "cpak": cpak,
            }
        )
    return in_maps


def kernel(x_temp, x_fea, mask, W_temp, b_temp, W_fea, b_fea, b, uw):
    nc = _get_module()
    in_maps = make_in_maps(
        x_temp, x_fea, mask, W_temp, b_temp, W_fea, b_fea, b, uw
    )
    res = run_bass_kernel_spmd(nc, in_maps, list(range(N_CORES)))
    return np.concatenate([res.results[k]["y"] for k in range(N_CORES)], axis=0)


# revision 60
# speedup vs baseline: 1.0257x; 1.0257x over previous
"""Self-contained Trainium2 Bass kernel for nn_AttLayer_model_5.

kernel(**inputs) takes the FULL unsharded inputs (B=64, T=2048, D=256, H=5),
shards the batch across 8 NeuronCores (data-parallel, 8 samples/core),
runs a Bass/Tile kernel via concourse.bass_utils.run_bass_kernel_spmd,
and gathers the full (64, 256) float32 output.

Math (per sample):
  temp  = x @ W_temp + b_temp          # (T,H), contraction over D
  fea   = xfea[:,None]*W_fea[0] + b_fea
  had   = tanh(temp) * tanh(fea)
  inter = had @ v, v = uw.sum(1)       # sum(b) shift dropped: softmax-invariant
  e     = exp(inter)                   # no max-subtraction: |inter| is bounded
                                       # by sum_h |v_h| ~ 0.5, fp32-safe
  wnum  = e * mask
  y     = (wnum @ x) / sum(wnum)       # (D,)

Device strategy (per core, 8 samples, x shard shipped bf16 = 8 MiB):
- x is cast to bf16 on host: every on-device x consumer (PE transposes at
  1 cyc/col vs 2 for fp32, projection, pooling matmuls at 1 cyc/col vs 4)
  runs at 16-bit PE rates and HBM traffic halves. End-to-end rel err vs
  the fp32 reference is ~2.4e-3 (gate is 2e-2): pooling error ~bf16 eps
  dampened by averaging; projection error further damped by
  d(inter)/d(temp) ~ 5e-3.
- x resident in SBUF, token-partition layout t = 16*p + c (contiguous DMA
  bursts), 4 quarter-DMAs per sample on the SP HWDGE queue in consumption
  order. Constants ride one packed [128,594] f32 tensor + one packed
  [12,4096] bf16 xfea/mask tensor on the ACT HWDGE queue -- any SWDGE
  (gpsimd) use costs ~0.4 ms/execute of setup on real HW.
- D-contraction transposes: both 128-row halves of a sample-stripe share
  one full-bank [128,1024] bf16 PSUM tile (transpose-mode matmuls), one
  psum->sbuf copy per sample-stripe, rotated DVE/DVE/ACT.
- Projection packs 4 samples per PSUM tile at partition offsets 32*j via
  matmul column tiling; biases ride ACT bias patterns; inter comes from a
  v-folding pattern matmul; softmax runs in an (8, T) layout; phases are
  group-major (samples 0-3 across all stripes, then 4-7), and the PE
  stream is software-pipelined one group deep: each group's projection/
  tanh chain is emitted behind the next group's independent transposes,
  hiding the ~us-scale real cross-engine semaphore latency that would
  otherwise head-of-line-block the in-order PE queue.
- Pooling: per 128-token chunk, wnum columns are PE-transposed and fed to
  M=1 bf16 matmuls (4 samples column-packed) accumulating fp32 in PSUM
  over 16 chunks, deferred one stripe to fill phase-B stalls; 1/sum(wnum)
  lands in a bulk per-partition-scalar scaled gather of the (pre-zeroed)
  accumulator banks -> 2 partition-strided y DMAs.
- The whole body is replicated K_IN times inside one program (constants
  loaded once, x re-read from HBM each iteration, y rewritten): one
  device execute performs K_IN genuine kernel executions, amortizing the
  per-execute axon dispatch cost so steady-state per-iteration hardware
  time is measurable from wall-clock slopes. x tiles, e/softmax tiles and
  output staging are double-buffered so iteration i+1's HBM reads and
  phase A overlap iteration i's phase B/pooling.

Cost-model per-core time: 119.5 us (fp32 baseline) -> 44.8 us/iteration.
Measured per-iteration HW time (8 cores concurrent, slope method):
~230-310 us depending on shared-terminal load; the x-read DMA floor
alone measures ~95-130 us/iteration in the same environment.
"""

import os
import sys
from contextlib import ExitStack

import numpy as np

for _p in ("/opt/trn_rl_repo", "/root/.axon_site/_ro/trn_rl_repo"):
    if os.path.isdir(_p) and _p not in sys.path:
        sys.path.insert(0, _p)
        break

import concourse.bass as bass
import concourse.mybir as mybir
import concourse.tile as tile
from concourse import bacc
from concourse.bass_utils import run_bass_kernel_spmd

F32 = mybir.dt.float32
BF16 = mybir.dt.bfloat16

N_CORES = 8
B = 64
B_LOC = B // N_CORES  # 8 samples per core
T = 2048
D = 256
H = 5
NC16 = T // 128
NQ = T // 512
K_IN = int(os.environ.get("BASS_K_IN", "64"))  # on-device kernel iterations per execute
# phase-bisection variants for HW attribution: full | dma | transp | proj
_VARIANT = os.environ.get("BASS_VARIANT", "full")
AF = mybir.ActivationFunctionType
ALU = mybir.AluOpType

# packed-constant column offsets in cpak [128, CPAK_COLS] f32
_IDENT0, _WT0, _VPAT0, _BT0, _BF0, _FPAT0, _PATG0 = 0, 128, 192, 208, 209, 210, 338
CPAK_COLS = 338 + 256


def _host_constants(W_temp, b_temp, W_fea, b_fea, uw):
    """Pure O(D*H + H^2) weight repacking on host into one tensor."""
    W_temp = np.asarray(W_temp, np.float32)
    b_temp = np.asarray(b_temp, np.float32)
    W_fea = np.asarray(W_fea, np.float32)
    b_fea = np.asarray(b_fea, np.float32)
    uw = np.asarray(uw, np.float32)

    v = uw.sum(axis=1)

    cpak = np.zeros((128, CPAK_COLS), np.float32)
    cpak[:, _IDENT0 : _IDENT0 + 128] = np.eye(128, dtype=np.float32)
    # wt: [128, 64], D halves at col offsets 0/32
    cpak[:, _WT0 : _WT0 + H] = W_temp[:128]
    cpak[:, _WT0 + 32 : _WT0 + 32 + H] = W_temp[128:]
    for s in range(B_LOC):
        g, j = divmod(s, 4)
        cpak[32 * j : 32 * j + H, _VPAT0 + 8 * g + s] = v
    for j in range(4):
        cpak[32 * j : 32 * j + H, _BT0] = b_temp
        cpak[32 * j : 32 * j + H, _BF0] = b_fea
        cpak[j, _FPAT0 + 32 * j : _FPAT0 + 32 * j + H] = W_fea[0]
    for g in range(2):
        for j in range(4):
            cpak[4 * g + j, _PATG0 + 128 * g + 32 * j] = 1.0
    return cpak


def _declare_io(nc):
    io = {}
    io["x"] = nc.dram_tensor("x", [B_LOC, T, D], BF16, kind="ExternalInput")
    # xm rows 0-3: xfea [4, 2T]; rows 4-11: mask-units [8, T] (cols T: pad)
    io["xm"] = nc.dram_tensor("xm", [12, 2 * T], BF16, kind="ExternalInput")
    io["cpak"] = nc.dram_tensor("cpak", [128, CPAK_COLS], F32, kind="ExternalInput")
    # unused pad input: forces HLO-hash/compile-cache misses so every
    # build of this program is compiled fresh (cache-buster, never read).
    # Keyed by K_IN and variant: program variants share I/O shapes, and a
    # stale HLO-hash hit would silently load the wrong NEFF.
    voff = {"full": 0, "dma": 1, "transp": 2, "proj": 3, "poolnf": 4, "poolend": 5}[
        _VARIANT
    ]
    io["pad"] = nc.dram_tensor(
        "pad", [1, 16 + K_IN + 512 * voff], F32, kind="ExternalInput"
    )
    io["y"] = nc.dram_tensor("y", [B_LOC, D], F32, kind="ExternalOutput")
    return io


class _Consts:
    pass


def _build_consts(nc, tc, io, ctx):
    """One-time loads/casts + pool creation shared by all body iterations."""
    C = _Consts()
    # All const loads ride the ACT HWDGE queue: any SWDGE (gpsimd) use
    # costs ~ms of per-execute setup on real HW (ucode library load +
    # software descriptor generation)
    cpool = ctx.enter_context(tc.tile_pool(name="consts", bufs=1))
    cpak = cpool.tile([128, CPAK_COLS], F32, name="cpak_sb")
    nc.scalar.dma_start(cpak[:], io["cpak"].ap()[:])
    xfea_h = cpool.tile([4, 2 * T], BF16, name="xfea_sb")
    nc.scalar.dma_start(xfea_h[:], io["xm"].ap()[0:4, :])
    masku_b = cpool.tile([B_LOC, T], BF16, name="masku_b")
    nc.scalar.dma_start(masku_b[:], io["xm"].ap()[4:12, 0:T])
    C.masku = cpool.tile([B_LOC, T], F32, name="masku_f")
    nc.vector.tensor_copy(C.masku[:], masku_b[:])

    C.ident_sb = cpak[:, _IDENT0 : _IDENT0 + 128]
    C.btpat = cpak[:, _BT0 : _BT0 + 1]
    C.bfpat = cpak[:, _BF0 : _BF0 + 1]
    C.patg = cpak[0:8, _PATG0 : _PATG0 + 256]
    C.xfea_h = xfea_h[:]

    # bf16 casts of the stationary matmul operands (fp32 can't pair with
    # bf16 on the PE)
    C.ident_h = cpool.tile([128, 128], BF16, name="ident_h")
    nc.vector.tensor_copy(C.ident_h[:], C.ident_sb)
    C.wt_h = cpool.tile([128, 64], BF16, name="wt_h")
    nc.vector.tensor_copy(C.wt_h[:], cpak[:, _WT0 : _WT0 + 64])
    C.vpat_h = cpool.tile([128, 16], BF16, name="vpat_h")
    nc.vector.tensor_copy(C.vpat_h[:], cpak[:, _VPAT0 : _VPAT0 + 16])
    C.fpat_h = cpool.tile([4, 128], BF16, name="fpat_h")
    nc.vector.tensor_copy(C.fpat_h[:], cpak[0:4, _FPAT0 : _FPAT0 + 128])

    # double-buffered across body iterations: x DMA and phase A of
    # iteration i+1 overlap phase B / pooling of iteration i
    C.xpool = ctx.enter_context(tc.tile_pool(name="xres", bufs=2))
    C.e_pool = ctx.enter_context(tc.tile_pool(name="epool", bufs=2))
    C.xtp_pool = ctx.enter_context(tc.tile_pool(name="xtp", bufs=3, space="PSUM"))
    C.xts_pool = ctx.enter_context(tc.tile_pool(name="xts", bufs=9))
    C.ttp_pool = ctx.enter_context(tc.tile_pool(name="ttp", bufs=1, space="PSUM"))
    C.fi_pool = ctx.enter_context(tc.tile_pool(name="fi", bufs=2, space="PSUM"))
    C.act_pool = ctx.enter_context(tc.tile_pool(name="acts", bufs=2))
    C.p3_pool = ctx.enter_context(tc.tile_pool(name="p3", bufs=1, space="PSUM"))
    C.out_pool = ctx.enter_context(tc.tile_pool(name="outp", bufs=2))
    return C


def _build_body(nc, tc, io, C, it):
    """One full kernel iteration: x HBM read -> compute -> y write."""
    mm = nc.tensor.matmul

    # x tiles: 4 quarter-DMAs per sample on the SP HWDGE queue, emitted in
    # the order the group-major phases consume them
    x_sb = [
        C.xpool.tile([128, NC16 * D], BF16, name=f"x{it}_{s}", tag=f"x{s}")
        for s in range(B_LOC)
    ]
    # one full-sample DMA each: the (c d) free dim is one contiguous 8 KiB
    # DRAM run per partition, so this emits 128 descriptors per sample
    # (4x fewer than quarter-DMAs; the real DMA path is descriptor-rate
    # bound). Arrival granularity is hidden by the cross-iteration double
    # buffering of x.
    for s in range(B_LOC):
        src = io["x"].ap()[s].rearrange("(p c) d -> p (c d)", c=NC16)
        nc.sync.dma_start(x_sb[s][:], src)

    do_proj = _VARIANT in ("proj", "full", "poolnf", "poolend")
    do_pool = _VARIANT in ("full", "poolnf", "poolend")
    defer_pool = _VARIANT != "poolend"  # poolend: all pooling after phase B
    do_finale = _VARIANT in ("full", "poolend")

    def y_stub():
        ystage = C.out_pool.tile([1, B_LOC * D], F32, name=f"ys{it}", tag="ystage")
        nc.vector.memset(ystage[:], 0.0)
        nc.sync.dma_start(io["y"].ap().rearrange("s d -> () (s d)"), ystage[:])

    if _VARIANT == "dma":
        y_stub()
        return

    # phase-3 accumulators: wtp and ypp0 share one bank-tile, ypp1 its own.
    # The banks are zeroed up front: the M=1 pooling matmuls only write the
    # 32*j sample bands, but the final scaled gather bulk-reads all rows.
    combo = C.p3_pool.tile([128, 512], F32, name=f"combo{it}", tag="combo")
    wtp = combo[:, 0:128]
    ypps = [
        combo[:, 128:384],
        C.p3_pool.tile([128, D], F32, name=f"ypp1_{it}", tag="ypp1"),
    ]
    recp = combo[:, 384:386]
    if do_pool:
        nc.vector.memset(combo[:, 128:386], 0.0)
        nc.vector.memset(ypps[1][:], 0.0)
    wts = C.out_pool.tile([128, 128], BF16, name=f"wts{it}", tag="wts")

    e_sb = C.e_pool.tile([B_LOC, T], F32, name=f"e{it}", tag="e_sb")
    den4_sb = C.e_pool.tile([B_LOC, NQ], F32, name=f"d4{it}", tag="den4")
    den_sb = C.e_pool.tile([B_LOC, 1], F32, name=f"d{it}", tag="den")
    rec_sb = C.e_pool.tile([B_LOC, 1], F32, name=f"r{it}", tag="rec")

    copy_flip = [0]

    def psum_to_sbuf(dst, src):
        # DVE copies bf16 ~1.6x faster than ACT; weight the rotation 2:1
        if copy_flip[0] % 3 != 2:
            nc.vector.tensor_copy(dst, src)
        else:
            nc.scalar.copy(dst, src)
        copy_flip[0] += 1

    # tanh(fea) stripes precomputed as PE filler during the x DMA wait
    tfs_all = {}

    def emit_tfs(q, g):
        fep = C.fi_pool.tile([128, 512], F32, name=f"fp{it}_{q}{g}", tag="fi")
        mm(fep[:], C.fpat_h[:], C.xfea_h[:, bass.ds(g * T + 512 * q, 512)])
        tfs = C.act_pool.tile(
            [128, 512], BF16, name=f"tf{it}_{q}{g}", tag="tfs", bufs=8
        )
        nc.scalar.activation(tfs[:], fep[:], AF.Tanh, bias=C.bfpat)
        tfs_all[(q, g)] = tfs

    tfs_todo = [(q, g) for g in range(2) for q in range(NQ)]

    def transp_group(q, g):
        """transposes + psum->sbuf(bf16) copies for one sample group.

        Both 128-row halves of the D contraction share one full-bank
        [128,1024] bf16 PSUM tile per sample, so 3 PSUM bufs hold 3
        samples in flight and each sample needs a single copy."""
        xts_h = {}
        for j in range(4):
            s = 4 * g + j
            xtp = C.xtp_pool.tile(
                [128, 1024], BF16, name=f"xp{it}_{q}{s}", tag="xtp"
            )
            for dh in range(2):
                for i in range(4):
                    c = 4 * q + i
                    mm(
                        xtp[:, 512 * dh + 128 * i : 512 * dh + 128 * (i + 1)],
                        x_sb[s][:, bass.ds(c * D + dh * 128, 128)],
                        C.ident_h[:],
                        is_transpose=True,
                        start=(dh == 0 and i == 0),
                        stop=(dh == 1 and i == 3),
                    )
            xts = C.xts_pool.tile(
                [128, 1024], BF16, name=f"xs{it}_{q}{s}", tag="xts"
            )
            psum_to_sbuf(xts[:], xtp[:])
            xts_h[j] = xts
            if do_proj and j == 1 and tfs_todo:
                emit_tfs(*tfs_todo.pop(0))
        if do_proj and tfs_todo:
            emit_tfs(*tfs_todo.pop(0))
        return xts_h

    def proj_mms(q, g, xts_h):
        ttp = C.ttp_pool.tile([128, 512], F32, name=f"tt{it}_{q}{g}", tag="ttp")
        for dh in range(2):
            for j in range(4):
                mm(
                    ttp[32 * j : 32 * j + 32, :],
                    C.wt_h[:, 32 * dh : 32 * dh + 32],
                    xts_h[j][:, 512 * dh : 512 * dh + 512],
                    start=(dh == 0),
                    stop=(dh == 1),
                    tile_position=(0, 32 * j),
                    skip_group_check=True,
                )
        return ttp

    def tanh_had_v(q, g, ttp):
        """tanh(temp), hadamard with precomputed tanh(fea), V-matmul."""
        tts = C.act_pool.tile([128, 512], F32, name=f"ts{it}_{q}{g}", tag="tts")
        nc.scalar.activation(tts[:], ttp[:], AF.Tanh, bias=C.btpat)
        had = C.act_pool.tile([128, 512], BF16, name=f"hd{it}_{q}{g}", tag="had")
        nc.vector.tensor_mul(had[:], tts[:], tfs_all[(q, g)][:])
        itp = C.fi_pool.tile([128, 512], F32, name=f"it{it}_{q}{g}", tag="fi")
        mm(itp[:8, :], C.vpat_h[:, 8 * g : 8 * g + 8], had[:])
        return itp

    def pool_wts(q):
        """w-transposes + psum->sbuf copy for stripe q, emitted right after
        its exp so the copy lands before the pooling MMs need it."""
        for i in range(4):
            c = 4 * q + i
            mm(
                wtp[:, 8 * c : 8 * c + 8],
                e_sb[:, 128 * c : 128 * (c + 1)],
                C.ident_sb[:8, :8],
                is_transpose=True,
                start=(c == 0),
                stop=(c == NC16 - 1),
                skip_group_check=True,
            )
        psum_to_sbuf(wts[:, 32 * q : 32 * (q + 1)], wtp[:, 32 * q : 32 * (q + 1)])

    def pool_mms(q):
        """packed bf16 pooling MMs for stripe q (one group behind pool_wts)."""
        for i in range(4):
            c = 4 * q + i
            for g in range(2):
                for j in range(4):
                    s = 4 * g + j
                    mm(
                        ypps[g][32 * j : 32 * j + 1, :],
                        wts[:, 8 * c + s : 8 * c + s + 1],
                        x_sb[s][:, bass.ds(c * D, D)],
                        start=(c == 0),
                        stop=(c == NC16 - 1),
                        tile_position=(0, 32 * j),
                        skip_group_check=True,
                    )

    def pool_stripe(q):
        pool_wts(q)
        pool_mms(q)

    def finish_group(q, g, xts_h):
        """projection + tanh/hadamard/V chain for a group whose transposes
        were emitted one group earlier (keeps independent transposes ahead
        of dependency-waiting matmuls in the in-order PE queue). Pooling
        lags further still -- wts transposes one group behind their exp,
        pooling MMs one group behind their wts copy -- so every PE
        instruction's dependencies are a full group old when it issues."""
        if g == 1 and do_pool and defer_pool and q >= 1:
            pool_wts(q - 1)
        ttp = proj_mms(q, g, xts_h)
        if g == 1 and do_pool and defer_pool and q >= 2:
            pool_mms(q - 2)
        itp = tanh_had_v(q, g, ttp)
        if g == 0:
            nc.vector.tensor_add(
                e_sb[:, bass.ds(512 * q, 512)],
                itp[:8, :],
                C.masku[:, bass.ds(512 * q, 512)],
            )
        else:
            inter = C.act_pool.tile([8, 512], F32, name=f"in{it}_{q}", tag="inter")
            nc.vector.tensor_add(
                inter[:], itp[:8, :], e_sb[:, bass.ds(512 * q, 512)]
            )
            nc.scalar.activation(
                e_sb[:, bass.ds(512 * q, 512)],
                inter[:],
                AF.Exp,
                accum_out=den4_sb[:, q : q + 1],
            )

    # ---- phases, PE stream software-pipelined one group deep:
    # group 0 (samples 0-3) across all stripes, then group 1 (4-7) ----
    pend = None
    for g in range(2):
        for q in range(NQ):
            xts_h = transp_group(q, g)
            if do_proj and pend is not None:
                finish_group(*pend)
            pend = (q, g, xts_h)
    if do_proj:
        finish_group(*pend)
    if not do_pool:
        y_stub()
        return
    if defer_pool:
        # tail: stripe NQ-2's MMs first (their wts copy is long done) to
        # cover exp(NQ-1)'s latency before the last wts transposes issue
        pool_mms(NQ - 2)
        pool_wts(NQ - 1)
        pool_mms(NQ - 1)
    else:
        for q in range(NQ):
            pool_stripe(q)
    if not do_finale:
        y_stub()
        return

    # ---- finale: denominators -> reciprocal patterns -> scaled gather ----
    nc.vector.tensor_reduce(
        den_sb[:], den4_sb[:], axis=mybir.AxisListType.X, op=ALU.add
    )
    nc.vector.reciprocal(rec_sb[:], den_sb[:])
    for g in range(2):
        mm(recp[:, g : g + 1], C.patg[:, 128 * g : 128 * (g + 1)], rec_sb[:])
    recs = C.out_pool.tile([128, 2], F32, name=f"rc{it}", tag="recs")
    nc.vector.tensor_copy(recs[:], recp[:])

    # scaled gather: one per-partition-scalar multiply per group (rows off
    # the 32*j sample bands scale by zero), then partition-strided y DMAs
    ystage = C.out_pool.tile([128, 2 * D], F32, name=f"ys{it}", tag="ystage")
    nc.vector.tensor_scalar_mul(ystage[:, 0:D], ypps[0][:, :], recs[:, 0:1])
    nc.scalar.mul(ystage[:, D : 2 * D], ypps[1][:, :], recs[:, 1:2])
    for g in range(2):
        src = ystage[:, g * D : (g + 1) * D].rearrange(
            "(j p) d -> j p d", p=32
        )[:, 0, :]
        nc.sync.dma_start(io["y"].ap()[4 * g : 4 * g + 4, :], src)


def _build(nc, tc, io, ctx):
    C = _build_consts(nc, tc, io, ctx)
    for it in range(K_IN):
        _build_body(nc, tc, io, C, it)


_MODULE_CACHE = {}


def _get_module():
    if "nc" not in _MODULE_CACHE:
        nc = bacc.Bacc("TRN2", target_bir_lowering=False, debug=False)
        io = _declare_io(nc)
        with tile.TileContext(nc) as tc:
            with ExitStack() as ctx:
                _build(nc, tc, io, ctx)
        nc.compile()
        _MODULE_CACHE["nc"] = nc
    return _MODULE_CACHE["nc"]


def make_in_maps(x_temp, x_fea, mask, W_temp, b_temp, W_fea, b_fea, b, uw):
    """Shard full inputs into per-core input maps (host-side, O(bytes))."""
    import ml_dtypes

    bf = ml_dtypes.bfloat16
    x_temp = np.ascontiguousarray(np.asarray(x_temp, np.float32).astype(bf))
    x_fea = np.asarray(x_fea, np.float32)
    masku = np.asarray(mask).astype(np.uint8)
    cpak = _host_constants(W_temp, b_temp, W_fea, b_fea, uw)

    in_maps = []
    for k in range(N_CORES):
        sl = slice(k * B_LOC, (k + 1) * B_LOC)
        # on-chip token order: free position 128*c + p <-> token 16*p + c
        xfea_p = (
            x_fea[sl].reshape(B_LOC, 128, NC16).swapaxes(1, 2).reshape(B_LOC, T)
        )
        xm = np.zeros((12, 2 * T), np.float32)
        xm[0:4] = xfea_p.reshape(2, 4, T).swapaxes(0, 1).reshape(4, 2 * T)
        xm[4:12, 0:T] = np.where(
            masku[sl].reshape(B_LOC, 128, NC16).swapaxes(1, 2).reshape(B_LOC, T)
            != 0,
            np.float32(0.0),
            np.float32(-1e30),
        )
        in_maps.append(
            {
                "pad": np.zeros(
                    (
                        1,
                        16
                        + K_IN
                        + 512
                        * {
                            "full": 0,
                            "dma": 1,
                            "transp": 2,
                            "proj": 3,
                            "poolnf": 4,
                            "poolend": 5,
                        }[_VARIANT],
                    ),
                    np.float32,
                ),
                "x": x_temp[sl],
                "xm": xm.astype(bf),
                "cpak": cpak,
            }
        )
    return in_maps


def kernel(x_temp, x_fea, mask, W_temp, b_temp, W_fea, b_fea, b, uw):
    nc = _get_module()
    in_maps = make_in_maps(
        x_temp, x_fea, mask, W_temp, b_temp, W_fea, b_fea, b, uw
    )
    res = run_bass_kernel_spmd(nc, in_maps, list(range(N_CORES)))
    return np.concatenate([res.results[k]["y"] for k in range(N_CORES)], axis=0)


# revision 61
# speedup vs baseline: 1.0679x; 1.0412x over previous
"""Self-contained Trainium2 Bass kernel for nn_AttLayer_model_5.

kernel(**inputs) takes the FULL unsharded inputs (B=64, T=2048, D=256, H=5),
shards the batch across 8 NeuronCores (data-parallel, 8 samples/core),
runs a Bass/Tile kernel via concourse.bass_utils.run_bass_kernel_spmd,
and gathers the full (64, 256) float32 output.

Math (per sample):
  temp  = x @ W_temp + b_temp          # (T,H), contraction over D
  fea   = xfea[:,None]*W_fea[0] + b_fea
  had   = tanh(temp) * tanh(fea)
  inter = had @ v, v = uw.sum(1)       # sum(b) shift dropped: softmax-invariant
  e     = exp(inter)                   # no max-subtraction: |inter| is bounded
                                       # by sum_h |v_h| ~ 0.5, fp32-safe
  wnum  = e * mask
  y     = (wnum @ x) / sum(wnum)       # (D,)

Device strategy (per core, 8 samples, x shard shipped bf16 = 8 MiB):
- x is cast to bf16 on host: every on-device x consumer (PE transposes at
  1 cyc/col vs 2 for fp32, projection, pooling matmuls at 1 cyc/col vs 4)
  runs at 16-bit PE rates and HBM traffic halves. End-to-end rel err vs
  the fp32 reference is ~2.4e-3 (gate is 2e-2): pooling error ~bf16 eps
  dampened by averaging; projection error further damped by
  d(inter)/d(temp) ~ 5e-3.
- x resident in SBUF, token-partition layout t = 16*p + c (contiguous DMA
  bursts), 4 quarter-DMAs per sample on the SP HWDGE queue in consumption
  order. Constants ride one packed [128,594] f32 tensor + one packed
  [12,4096] bf16 xfea/mask tensor on the ACT HWDGE queue -- any SWDGE
  (gpsimd) use costs ~0.4 ms/execute of setup on real HW.
- D-contraction transposes: both 128-row halves of a sample-stripe share
  one full-bank [128,1024] bf16 PSUM tile (transpose-mode matmuls), one
  psum->sbuf copy per sample-stripe, rotated DVE/DVE/ACT.
- Projection packs 4 samples per PSUM tile at partition offsets 32*j via
  matmul column tiling; biases ride ACT bias patterns; inter comes from a
  v-folding pattern matmul; softmax runs in an (8, T) layout; phases are
  group-major (samples 0-3 across all stripes, then 4-7), and the PE
  stream is software-pipelined one group deep: each group's projection/
  tanh chain is emitted behind the next group's independent transposes,
  hiding the ~us-scale real cross-engine semaphore latency that would
  otherwise head-of-line-block the in-order PE queue.
- Pooling: per 128-token chunk, wnum columns are PE-transposed and fed to
  M=1 bf16 matmuls (4 samples column-packed) accumulating fp32 in PSUM
  over 16 chunks, deferred one stripe to fill phase-B stalls; 1/sum(wnum)
  lands in a bulk per-partition-scalar scaled gather of the (pre-zeroed)
  accumulator banks -> 2 partition-strided y DMAs.
- The whole body is replicated K_IN times inside one program (constants
  loaded once, x re-read from HBM each iteration, y rewritten): one
  device execute performs K_IN genuine kernel executions, amortizing the
  per-execute axon dispatch cost so steady-state per-iteration hardware
  time is measurable from wall-clock slopes. x tiles, e/softmax tiles and
  output staging are double-buffered so iteration i+1's HBM reads and
  phase A overlap iteration i's phase B/pooling.

Cost-model per-core time: 119.5 us (fp32 baseline) -> 44.8 us/iteration.
Measured per-iteration HW time (8 cores concurrent, slope method):
~230-310 us depending on shared-terminal load; the x-read DMA floor
alone measures ~95-130 us/iteration in the same environment.
"""

import os
import sys
from contextlib import ExitStack

import numpy as np

for _p in ("/opt/trn_rl_repo", "/root/.axon_site/_ro/trn_rl_repo"):
    if os.path.isdir(_p) and _p not in sys.path:
        sys.path.insert(0, _p)
        break

import concourse.bass as bass
import concourse.mybir as mybir
import concourse.tile as tile
from concourse import bacc
from concourse.bass_utils import run_bass_kernel_spmd

F32 = mybir.dt.float32
BF16 = mybir.dt.bfloat16

N_CORES = 8
B = 64
B_LOC = B // N_CORES  # 8 samples per core
T = 2048
D = 256
H = 5
NC16 = T // 128
NQ = T // 512
K_IN = int(os.environ.get("BASS_K_IN", "128"))  # on-device kernel iterations per execute
# phase-bisection variants for HW attribution: full | dma | transp | proj
_VARIANT = os.environ.get("BASS_VARIANT", "full")
AF = mybir.ActivationFunctionType
ALU = mybir.AluOpType

# packed-constant column offsets in cpak [128, CPAK_COLS] f32
_IDENT0, _WT0, _VPAT0, _BT0, _BF0, _FPAT0, _PATG0 = 0, 128, 192, 208, 209, 210, 338
CPAK_COLS = 338 + 256


def _host_constants(W_temp, b_temp, W_fea, b_fea, uw):
    """Pure O(D*H + H^2) weight repacking on host into one tensor."""
    W_temp = np.asarray(W_temp, np.float32)
    b_temp = np.asarray(b_temp, np.float32)
    W_fea = np.asarray(W_fea, np.float32)
    b_fea = np.asarray(b_fea, np.float32)
    uw = np.asarray(uw, np.float32)

    v = uw.sum(axis=1)

    cpak = np.zeros((128, CPAK_COLS), np.float32)
    cpak[:, _IDENT0 : _IDENT0 + 128] = np.eye(128, dtype=np.float32)
    # wt: [128, 64], D halves at col offsets 0/32
    cpak[:, _WT0 : _WT0 + H] = W_temp[:128]
    cpak[:, _WT0 + 32 : _WT0 + 32 + H] = W_temp[128:]
    for s in range(B_LOC):
        g, j = divmod(s, 4)
        cpak[32 * j : 32 * j + H, _VPAT0 + 8 * g + s] = v
    for j in range(4):
        cpak[32 * j : 32 * j + H, _BT0] = b_temp
        cpak[32 * j : 32 * j + H, _BF0] = b_fea
        cpak[j, _FPAT0 + 32 * j : _FPAT0 + 32 * j + H] = W_fea[0]
    for g in range(2):
        for j in range(4):
            cpak[4 * g + j, _PATG0 + 128 * g + 32 * j] = 1.0
    return cpak


def _declare_io(nc):
    io = {}
    io["x"] = nc.dram_tensor("x", [B_LOC, T, D], BF16, kind="ExternalInput")
    # xm rows 0-3: xfea [4, 2T]; rows 4-11: mask-units [8, T] (cols T: pad)
    io["xm"] = nc.dram_tensor("xm", [12, 2 * T], BF16, kind="ExternalInput")
    io["cpak"] = nc.dram_tensor("cpak", [128, CPAK_COLS], F32, kind="ExternalInput")
    # unused pad input: forces HLO-hash/compile-cache misses so every
    # build of this program is compiled fresh (cache-buster, never read).
    # Keyed by K_IN and variant: program variants share I/O shapes, and a
    # stale HLO-hash hit would silently load the wrong NEFF.
    voff = {"full": 0, "dma": 1, "transp": 2, "proj": 3, "poolnf": 4, "poolend": 5}[
        _VARIANT
    ]
    io["pad"] = nc.dram_tensor(
        "pad", [1, 16 + K_IN + 512 * voff], F32, kind="ExternalInput"
    )
    io["y"] = nc.dram_tensor("y", [B_LOC, D], F32, kind="ExternalOutput")
    return io


class _Consts:
    pass


def _build_consts(nc, tc, io, ctx):
    """One-time loads/casts + pool creation shared by all body iterations."""
    C = _Consts()
    # All const loads ride the ACT HWDGE queue: any SWDGE (gpsimd) use
    # costs ~ms of per-execute setup on real HW (ucode library load +
    # software descriptor generation)
    cpool = ctx.enter_context(tc.tile_pool(name="consts", bufs=1))
    cpak = cpool.tile([128, CPAK_COLS], F32, name="cpak_sb")
    nc.scalar.dma_start(cpak[:], io["cpak"].ap()[:])
    xfea_h = cpool.tile([4, 2 * T], BF16, name="xfea_sb")
    nc.scalar.dma_start(xfea_h[:], io["xm"].ap()[0:4, :])
    masku_b = cpool.tile([B_LOC, T], BF16, name="masku_b")
    nc.scalar.dma_start(masku_b[:], io["xm"].ap()[4:12, 0:T])
    C.masku = cpool.tile([B_LOC, T], F32, name="masku_f")
    nc.vector.tensor_copy(C.masku[:], masku_b[:])

    C.ident_sb = cpak[:, _IDENT0 : _IDENT0 + 128]
    C.btpat = cpak[:, _BT0 : _BT0 + 1]
    C.bfpat = cpak[:, _BF0 : _BF0 + 1]
    C.patg = cpak[0:8, _PATG0 : _PATG0 + 256]
    C.xfea_h = xfea_h[:]

    # bf16 casts of the stationary matmul operands (fp32 can't pair with
    # bf16 on the PE)
    C.ident_h = cpool.tile([128, 128], BF16, name="ident_h")
    nc.vector.tensor_copy(C.ident_h[:], C.ident_sb)
    C.wt_h = cpool.tile([128, 64], BF16, name="wt_h")
    nc.vector.tensor_copy(C.wt_h[:], cpak[:, _WT0 : _WT0 + 64])
    C.vpat_h = cpool.tile([128, 16], BF16, name="vpat_h")
    nc.vector.tensor_copy(C.vpat_h[:], cpak[:, _VPAT0 : _VPAT0 + 16])
    C.fpat_h = cpool.tile([4, 128], BF16, name="fpat_h")
    nc.vector.tensor_copy(C.fpat_h[:], cpak[0:4, _FPAT0 : _FPAT0 + 128])

    # double-buffered across body iterations: x DMA and phase A of
    # iteration i+1 overlap phase B / pooling of iteration i
    C.xpool = ctx.enter_context(tc.tile_pool(name="xres", bufs=2))
    C.e_pool = ctx.enter_context(tc.tile_pool(name="epool", bufs=2))
    C.xtp_pool = ctx.enter_context(tc.tile_pool(name="xtp", bufs=3, space="PSUM"))
    C.xts_pool = ctx.enter_context(tc.tile_pool(name="xts", bufs=9))
    C.ttp_pool = ctx.enter_context(tc.tile_pool(name="ttp", bufs=1, space="PSUM"))
    C.fi_pool = ctx.enter_context(tc.tile_pool(name="fi", bufs=2, space="PSUM"))
    C.act_pool = ctx.enter_context(tc.tile_pool(name="acts", bufs=2))
    C.p3_pool = ctx.enter_context(tc.tile_pool(name="p3", bufs=1, space="PSUM"))
    C.out_pool = ctx.enter_context(tc.tile_pool(name="outp", bufs=2))
    return C


def _build_body(nc, tc, io, C, it):
    """One full kernel iteration: x HBM read -> compute -> y write."""
    mm = nc.tensor.matmul

    # x tiles: 4 quarter-DMAs per sample on the SP HWDGE queue, emitted in
    # the order the group-major phases consume them
    x_sb = [
        C.xpool.tile([128, NC16 * D], BF16, name=f"x{it}_{s}", tag=f"x{s}")
        for s in range(B_LOC)
    ]
    # one full-sample DMA each: the (c d) free dim is one contiguous 8 KiB
    # DRAM run per partition, so this emits 128 descriptors per sample
    # (4x fewer than quarter-DMAs; the real DMA path is descriptor-rate
    # bound). Arrival granularity is hidden by the cross-iteration double
    # buffering of x.
    for s in range(B_LOC):
        src = io["x"].ap()[s].rearrange("(p c) d -> p (c d)", c=NC16)
        nc.sync.dma_start(x_sb[s][:], src)

    do_proj = _VARIANT in ("proj", "full", "poolnf", "poolend")
    do_pool = _VARIANT in ("full", "poolnf", "poolend")
    defer_pool = _VARIANT != "poolend"  # poolend: all pooling after phase B
    do_finale = _VARIANT in ("full", "poolend")

    def y_stub():
        ystage = C.out_pool.tile([1, B_LOC * D], F32, name=f"ys{it}", tag="ystage")
        nc.vector.memset(ystage[:], 0.0)
        nc.sync.dma_start(io["y"].ap().rearrange("s d -> () (s d)"), ystage[:])

    if _VARIANT == "dma":
        y_stub()
        return

    # phase-3 accumulators: wtp and ypp0 share one bank-tile, ypp1 its own.
    # The banks are zeroed up front: the M=1 pooling matmuls only write the
    # 32*j sample bands, but the final scaled gather bulk-reads all rows.
    combo = C.p3_pool.tile([128, 512], F32, name=f"combo{it}", tag="combo")
    wtp = combo[:, 0:128]
    ypps = [
        combo[:, 128:384],
        C.p3_pool.tile([128, D], F32, name=f"ypp1_{it}", tag="ypp1"),
    ]
    recp = combo[:, 384:386]
    if do_pool:
        nc.vector.memset(combo[:, 128:386], 0.0)
        nc.vector.memset(ypps[1][:], 0.0)
    wts = C.out_pool.tile([128, 128], BF16, name=f"wts{it}", tag="wts")

    e_sb = C.e_pool.tile([B_LOC, T], F32, name=f"e{it}", tag="e_sb")
    den4_sb = C.e_pool.tile([B_LOC, NQ], F32, name=f"d4{it}", tag="den4")
    den_sb = C.e_pool.tile([B_LOC, 1], F32, name=f"d{it}", tag="den")
    rec_sb = C.e_pool.tile([B_LOC, 1], F32, name=f"r{it}", tag="rec")

    copy_flip = [0]

    def psum_to_sbuf(dst, src):
        # DVE copies bf16 ~1.6x faster than ACT; weight the rotation 2:1
        if copy_flip[0] % 3 != 2:
            nc.vector.tensor_copy(dst, src)
        else:
            nc.scalar.copy(dst, src)
        copy_flip[0] += 1

    # tanh(fea) stripes precomputed as PE filler during the x DMA wait
    tfs_all = {}

    def emit_tfs(q, g):
        fep = C.fi_pool.tile([128, 512], F32, name=f"fp{it}_{q}{g}", tag="fi")
        mm(fep[:], C.fpat_h[:], C.xfea_h[:, bass.ds(g * T + 512 * q, 512)])
        tfs = C.act_pool.tile(
            [128, 512], BF16, name=f"tf{it}_{q}{g}", tag="tfs", bufs=8
        )
        nc.scalar.activation(tfs[:], fep[:], AF.Tanh, bias=C.bfpat)
        tfs_all[(q, g)] = tfs

    tfs_todo = [(q, g) for g in range(2) for q in range(NQ)]

    def transp_group(q, g):
        """transposes + psum->sbuf(bf16) copies for one sample group.

        Both 128-row halves of the D contraction share one full-bank
        [128,1024] bf16 PSUM tile per sample, so 3 PSUM bufs hold 3
        samples in flight and each sample needs a single copy."""
        xts_h = {}
        for j in range(4):
            s = 4 * g + j
            xtp = C.xtp_pool.tile(
                [128, 1024], BF16, name=f"xp{it}_{q}{s}", tag="xtp"
            )
            for dh in range(2):
                for i in range(4):
                    c = 4 * q + i
                    mm(
                        xtp[:, 512 * dh + 128 * i : 512 * dh + 128 * (i + 1)],
                        x_sb[s][:, bass.ds(c * D + dh * 128, 128)],
                        C.ident_h[:],
                        is_transpose=True,
                        start=(dh == 0 and i == 0),
                        stop=(dh == 1 and i == 3),
                    )
            xts = C.xts_pool.tile(
                [128, 1024], BF16, name=f"xs{it}_{q}{s}", tag="xts"
            )
            psum_to_sbuf(xts[:], xtp[:])
            xts_h[j] = xts
            if do_proj and j == 1 and tfs_todo:
                emit_tfs(*tfs_todo.pop(0))
        if do_proj and tfs_todo:
            emit_tfs(*tfs_todo.pop(0))
        return xts_h

    def proj_mms(q, g, xts_h):
        ttp = C.ttp_pool.tile([128, 512], F32, name=f"tt{it}_{q}{g}", tag="ttp")
        for dh in range(2):
            for j in range(4):
                mm(
                    ttp[32 * j : 32 * j + 32, :],
                    C.wt_h[:, 32 * dh : 32 * dh + 32],
                    xts_h[j][:, 512 * dh : 512 * dh + 512],
                    start=(dh == 0),
                    stop=(dh == 1),
                    tile_position=(0, 32 * j),
                    skip_group_check=True,
                )
        return ttp

    def tanh_had_v(q, g, ttp):
        """tanh(temp), hadamard with precomputed tanh(fea), V-matmul."""
        tts = C.act_pool.tile([128, 512], F32, name=f"ts{it}_{q}{g}", tag="tts")
        nc.scalar.activation(tts[:], ttp[:], AF.Tanh, bias=C.btpat)
        had = C.act_pool.tile([128, 512], BF16, name=f"hd{it}_{q}{g}", tag="had")
        nc.vector.tensor_mul(had[:], tts[:], tfs_all[(q, g)][:])
        itp = C.fi_pool.tile([128, 512], F32, name=f"it{it}_{q}{g}", tag="fi")
        mm(itp[:8, :], C.vpat_h[:, 8 * g : 8 * g + 8], had[:])
        return itp

    def pool_wts(q):
        """w-transposes + psum->sbuf copy for stripe q, emitted right after
        its exp so the copy lands before the pooling MMs need it."""
        for i in range(4):
            c = 4 * q + i
            mm(
                wtp[:, 8 * c : 8 * c + 8],
                e_sb[:, 128 * c : 128 * (c + 1)],
                C.ident_sb[:8, :8],
                is_transpose=True,
                start=(c == 0),
                stop=(c == NC16 - 1),
                skip_group_check=True,
            )
        psum_to_sbuf(wts[:, 32 * q : 32 * (q + 1)], wtp[:, 32 * q : 32 * (q + 1)])

    def pool_mms(q):
        """packed bf16 pooling MMs for stripe q (one group behind pool_wts)."""
        for i in range(4):
            c = 4 * q + i
            for g in range(2):
                for j in range(4):
                    s = 4 * g + j
                    mm(
                        ypps[g][32 * j : 32 * j + 1, :],
                        wts[:, 8 * c + s : 8 * c + s + 1],
                        x_sb[s][:, bass.ds(c * D, D)],
                        start=(c == 0),
                        stop=(c == NC16 - 1),
                        tile_position=(0, 32 * j),
                        skip_group_check=True,
                    )

    def pool_stripe(q):
        pool_wts(q)
        pool_mms(q)

    def finish_group(q, g, xts_h):
        """projection + tanh/hadamard/V chain for a group whose transposes
        were emitted one group earlier (keeps independent transposes ahead
        of dependency-waiting matmuls in the in-order PE queue). Pooling
        lags further still -- wts transposes one group behind their exp,
        pooling MMs one group behind their wts copy -- so every PE
        instruction's dependencies are a full group old when it issues."""
        if g == 1 and do_pool and defer_pool and q >= 1:
            pool_wts(q - 1)
        ttp = proj_mms(q, g, xts_h)
        if g == 1 and do_pool and defer_pool and q >= 2:
            pool_mms(q - 2)
        itp = tanh_had_v(q, g, ttp)
        if g == 0:
            nc.vector.tensor_add(
                e_sb[:, bass.ds(512 * q, 512)],
                itp[:8, :],
                C.masku[:, bass.ds(512 * q, 512)],
            )
        else:
            inter = C.act_pool.tile([8, 512], F32, name=f"in{it}_{q}", tag="inter")
            nc.vector.tensor_add(
                inter[:], itp[:8, :], e_sb[:, bass.ds(512 * q, 512)]
            )
            nc.scalar.activation(
                e_sb[:, bass.ds(512 * q, 512)],
                inter[:],
                AF.Exp,
                accum_out=den4_sb[:, q : q + 1],
            )

    # ---- phases, PE stream software-pipelined one group deep:
    # group 0 (samples 0-3) across all stripes, then group 1 (4-7) ----
    pend = None
    for g in range(2):
        for q in range(NQ):
            xts_h = transp_group(q, g)
            if do_proj and pend is not None:
                finish_group(*pend)
            pend = (q, g, xts_h)
    if do_proj:
        finish_group(*pend)
    if not do_pool:
        y_stub()
        return
    if defer_pool:
        # tail: stripe NQ-2's MMs first (their wts copy is long done) to
        # cover exp(NQ-1)'s latency before the last wts transposes issue
        pool_mms(NQ - 2)
        pool_wts(NQ - 1)
        pool_mms(NQ - 1)
    else:
        for q in range(NQ):
            pool_stripe(q)
    if not do_finale:
        y_stub()
        return

    # ---- finale: denominators -> reciprocal patterns -> scaled gather ----
    nc.vector.tensor_reduce(
        den_sb[:], den4_sb[:], axis=mybir.AxisListType.X, op=ALU.add
    )
    nc.vector.reciprocal(rec_sb[:], den_sb[:])
    for g in range(2):
        mm(recp[:, g : g + 1], C.patg[:, 128 * g : 128 * (g + 1)], rec_sb[:])
    recs = C.out_pool.tile([128, 2], F32, name=f"rc{it}", tag="recs")
    nc.vector.tensor_copy(recs[:], recp[:])

    # scaled gather: one per-partition-scalar multiply per group (rows off
    # the 32*j sample bands scale by zero), then partition-strided y DMAs
    ystage = C.out_pool.tile([128, 2 * D], F32, name=f"ys{it}", tag="ystage")
    nc.vector.tensor_scalar_mul(ystage[:, 0:D], ypps[0][:, :], recs[:, 0:1])
    nc.scalar.mul(ystage[:, D : 2 * D], ypps[1][:, :], recs[:, 1:2])
    for g in range(2):
        src = ystage[:, g * D : (g + 1) * D].rearrange(
            "(j p) d -> j p d", p=32
        )[:, 0, :]
        nc.sync.dma_start(io["y"].ap()[4 * g : 4 * g + 4, :], src)


def _build(nc, tc, io, ctx):
    C = _build_consts(nc, tc, io, ctx)
    for it in range(K_IN):
        _build_body(nc, tc, io, C, it)


_MODULE_CACHE = {}


def _get_module():
    if "nc" not in _MODULE_CACHE:
        nc = bacc.Bacc("TRN2", target_bir_lowering=False, debug=False)
        io = _declare_io(nc)
        with tile.TileContext(nc) as tc:
            with ExitStack() as ctx:
                _build(nc, tc, io, ctx)
        nc.compile()
        _MODULE_CACHE["nc"] = nc
    return _MODULE_CACHE["nc"]


def make_in_maps(x_temp, x_fea, mask, W_temp, b_temp, W_fea, b_fea, b, uw):
    """Shard full inputs into per-core input maps (host-side, O(bytes))."""
    import ml_dtypes

    bf = ml_dtypes.bfloat16
    x_temp = np.ascontiguousarray(np.asarray(x_temp, np.float32).astype(bf))
    x_fea = np.asarray(x_fea, np.float32)
    masku = np.asarray(mask).astype(np.uint8)
    cpak = _host_constants(W_temp, b_temp, W_fea, b_fea, uw)

    in_maps = []
    for k in range(N_CORES):
        sl = slice(k * B_LOC, (k + 1) * B_LOC)
        # on-chip token order: free position 128*c + p <-> token 16*p + c
        xfea_p = (
            x_fea[sl].reshape(B_LOC, 128, NC16).swapaxes(1, 2).reshape(B_LOC, T)
        )
        xm = np.zeros((12, 2 * T), np.float32)
        xm[0:4] = xfea_p.reshape(2, 4, T).swapaxes(0, 1).reshape(4, 2 * T)
        xm[4:12, 0:T] = np.where(
            masku[sl].reshape(B_LOC, 128, NC16).swapaxes(1, 2).reshape(B_LOC, T)
            != 0,
            np.float32(0.0),
            np.float32(-1e30),
        )
        in_maps.append(
            {
                "pad": np.zeros(
                    (
                        1,
                        16
                        + K_IN
                        + 512
                        * {
                            "full": 0,
                            "dma": 1,
                            "transp": 2,
                            "proj": 3,
                            "poolnf": 4,
                            "poolend": 5,
                        }[_VARIANT],
                    ),
                    np.float32,
                ),
                "x": x_temp[sl],
                "xm": xm.astype(bf),
                "cpak": cpak,
            }
        )
    return in_maps


def kernel(x_temp, x_fea, mask, W_temp, b_temp, W_fea, b_fea, b, uw):
    nc = _get_module()
    in_maps = make_in_maps(
        x_temp, x_fea, mask, W_temp, b_temp, W_fea, b_fea, b, uw
    )
    res = run_bass_kernel_spmd(nc, in_maps, list(range(N_CORES)))
    return np.concatenate([res.results[k]["y"] for k in range(N_CORES)], axis=0)
